# revision 11
# baseline (speedup 1.0000x reference)
"""CTRNN forward kernel for Trainium2 (8 NeuronCores, data-parallel over batch).

Reference computation (per step t, dt=0.02):
    h = h*(1-dt) + dt*(tanh(h) @ J.T + v_t @ Bmat.T)
    out_t = tanh(h) @ W_ro.T

Design (v6): scratch-bank accumulation to shorten the serial chain.
  - Per core: B_LOC=16 batch rows, hT layout (hidden on partitions, 4 row
    blocks of 128; batch on free dim). State S = 64*h in PSUM (fp16 guard).
  - v3's chain was ACT -> stt -> MMs -> ACT (the DVE op pre-writes the
    decayed state, so the MMs must wait for it). v6 instead accumulates the
    step's matmul contributions into a per-group SCRATCH psum bank Z
    (first MM start=True claims/overwrites), so the J-MMs launch right
    after ACT(t-1); then ONE DVE scalar_tensor_tensor per group computes
    S_new = 0.98*S_old + Z into the next state set. The ACT<->DVE
    same-bank serialization (both read S_old) hides behind the MM phase.
  - The input term 64*dt*Bmat*v_t enters Z via K=1 matmuls (lhsT = Bmat
    row block, rhs = v_t row from SBUF) placed before the J-MMs; their
    weight loads run during the previous step's tail (off-chain).
  - 2 rotating state sets x 2 bank groups + 2 Z banks + readout + junk =
    8 PSUM banks exactly.
  - 16 fp16 J-MMs/step in the v3-tuned PI issue order; ACTs produce the
    y ring (32-step fp16 ring), readout batched per 32 steps as in v3.
"""

import math
import sys

import numpy as np

sys.path.insert(0, "/opt/trn_rl_repo")

DT = 0.02
DECAY = 1.0 - DT          # 0.98
HSCALE = 64.0             # h kept as 64*h in PSUM (fp16 subnormal guard)
HIDDEN = 512
BATCH = 128
T_FULL = 1024
N_CORES = 8
B_LOC = BATCH // N_CORES  # 16
CB = HIDDEN // 128        # 4 row blocks / y chunks

# MM issue order (block b, chunk c); groups: bank A = blocks {0,1},
# bank B = blocks {2,3}. v3's steady-state-search order.
PI = [
    (1, 1), (0, 1), (1, 0), (2, 0), (2, 1), (0, 0), (0, 3), (1, 2),
    (0, 2), (1, 3), (3, 0), (3, 2), (3, 1), (3, 3), (2, 3), (2, 2),
]


def build_nc(T=T_FULL, ro=32):
    import concourse.bass as bass
    import concourse.tile as tile
    from concourse import bacc, mybir

    f32 = mybir.dt.float32
    f16 = mybir.dt.float16
    nc = bacc.Bacc()

    jt_h = nc.declare_dram_parameter("JT", [HIDDEN, HIDDEN], f16, isOutput=False)
    brow_h = nc.declare_dram_parameter("brow", [1, HIDDEN], f16, isOutput=False)
    velt_h = nc.declare_dram_parameter("velT", [1, T * B_LOC], f16, isOutput=False)
    wrt_h = nc.declare_dram_parameter("wrt", [128, CB], f16, isOutput=False)
    out_h = nc.declare_dram_parameter("out", [1, T * B_LOC], f32, isOutput=True)

    nro = (T + ro - 1) // ro
    rosz = ro * B_LOC  # 512 = one PSUM bank of fp32

    with tile.TileContext(nc) as tc:
        with (
            tc.tile_pool(name="singles", bufs=1) as singles,
            tc.tile_pool(name="ybp", bufs=2) as ybp,
            tc.tile_pool(name="osbp", bufs=2) as osbp,
            tc.tile_pool(name="psum", bufs=1, space="PSUM") as pp,
        ):
            # ---- weights staging ----
            jt = singles.tile([128, CB, HIDDEN], f16, tag="jt")  # (64*dt*J)^T
            nc.sync.dma_start(out=jt, in_=jt_h.rearrange("(c p) i -> p c i", p=128))
            browt = singles.tile([1, HIDDEN], f16, tag="browt")  # 64*dt*Bmat
            nc.sync.dma_start(out=browt, in_=brow_h[:, :])
            velt = singles.tile([1, T * B_LOC], f16, tag="velt")
            nc.sync.dma_start(out=velt, in_=velt_h[:, :])
            wrt = singles.tile([128, CB], f16, tag="wrt")
            nc.sync.dma_start(out=wrt, in_=wrt_h[:, :])

            y0 = singles.tile([128, B_LOC], f16, tag="y0")
            nc.vector.memset(y0.bitcast(f32), 0.0)

            # state sets in SBUF: xy[s][g] = [128, 32] (blocks 2g, 2g+1)
            xy = [
                [
                    singles.tile(
                        [128, 2 * B_LOC], f32, tag=f"s{s}{g}", name=f"sb_s{s}{g}"
                    )
                    for g in range(2)
                ]
                for s in range(2)
            ]
            for s in range(2):
                for g in range(2):
                    nc.vector.memset(xy[s][g], 0.0)
            # per-group scratch accumulators for the current step's MMs
            Z = [
                pp.tile([128, 2 * B_LOC], f32, tag=f"z{g}", name=f"psum_z{g}")
                for g in range(2)
            ]
            pjunk = pp.tile([1, 8], f32, tag="junk", name="psum_junk")

            def absorb(src):
                if src.dtype != f32:
                    src = src.bitcast(f32)
                nc.tensor.matmul(
                    out=pjunk[0:1, 0:1],
                    lhsT=src,
                    rhs=src,
                    start=True,
                    stop=True,
                    skip_group_check=True,
                )

            absorb(jt[0:1, 0, 0:2])
            absorb(browt[0:1, 0:2])
            absorb(velt[0:1, 0:2])
            absorb(wrt[0:1, 0:2])

            def emit_readout(k, ytile):
                pro = pp.tile([1, rosz], f32, tag="ro", bufs=1, name="psum_ro")
                for c in range(CB):
                    nc.tensor.matmul(
                        out=pro,
                        lhsT=wrt[:, c : c + 1],
                        rhs=ytile[:, c, :, :].rearrange("p t b -> p (t b)"),
                        start=(c == 0),
                        stop=(c == CB - 1),
                        skip_group_check=True,
                    )
                osb = osbp.tile([1, rosz], f32, tag="osb", name="out_sb")
                nc.vector.tensor_copy(osb, pro)
                nc.sync.dma_start(
                    out=out_h[0:1, k * rosz : (k + 1) * rosz], in_=osb
                )

            yb_cur = None
            yb_prev = None
            for t in range(T):
                rob, rj = divmod(t, ro)

                if rj == 0:
                    yb_prev = yb_cur
                    yb_cur = ybp.tile([128, CB, ro, B_LOC], f16, tag="yb")

                # batched readout of the previous 32-step block
                if rj == 4 and rob >= 1:
                    emit_readout(rob - 1, yb_prev)

                if t == 0:
                    ysl = lambda c: y0
                elif rj == 0:
                    ysl = lambda c: yb_prev[:, c, ro - 1, :]
                else:
                    ysl = lambda c, _s=rj - 1: yb_cur[:, c, _s, :]

                # ---- the step ----
                # input term first: claims Z (start=True on each bank's
                # first MM clears the bank, the second block overwrites its
                # own still-clear region). These depend only on velt + the
                # previous merge's read of Z -> they run during step t-1's
                # tail, keeping their weight loads off the chain.
                for b in range(4):
                    nc.tensor.matmul(
                        out=Z[b // 2][:, 16 * (b % 2) : 16 * (b % 2) + 16],
                        lhsT=browt[0:1, 128 * b : 128 * (b + 1)],
                        rhs=velt[0:1, t * B_LOC : (t + 1) * B_LOC],
                        start=(b % 2 == 0),
                        stop=False,
                        skip_group_check=True,
                    )
                # recurrent MMs (need y(t-1); start right after ACT(t-1))
                for b, c in PI:
                    nc.tensor.matmul(
                        out=Z[b // 2][:, 16 * (b % 2) : 16 * (b % 2) + 16],
                        lhsT=jt[:, c, 128 * b : 128 * (b + 1)],
                        rhs=ysl(c),
                        start=False,
                        stop=False,
                        skip_group_check=True,
                    )
                # merge: S_new = 0.98*S_old + Z   (per group, on DVE)
                S_new = xy[t % 2]
                S_old = xy[(t + 1) % 2]
                for g in range(2):
                    nc.vector.scalar_tensor_tensor(
                        out=S_new[g],
                        in0=S_old[g],
                        scalar=float(DECAY),
                        in1=Z[g],
                        op0=mybir.AluOpType.mult,
                        op1=mybir.AluOpType.add,
                    )
                # tanh into the y ring
                for g in range(2):
                    nc.scalar.activation(
                        out=yb_cur[:, 2 * g : 2 * g + 2, rj, :],
                        in_=S_new[g].rearrange("p (c b) -> p c b", b=B_LOC),
                        func=mybir.ActivationFunctionType.Tanh,
                        scale=1.0 / HSCALE,
                    )

            emit_readout(nro - 1, yb_cur)

    nc.compile()
    return nc


_NC_CACHE = {}


def _get_nc(**kw):
    key = tuple(sorted(kw.items()))
    if key not in _NC_CACHE:
        _NC_CACHE[key] = build_nc(**kw)
    return _NC_CACHE[key]


def make_in_maps(vel, J, Bmat, W_ro):
    vel = np.asarray(vel, dtype=np.float32)[:, :, 0]          # [B, T]
    J = np.asarray(J, dtype=np.float32)
    Bmat = np.asarray(Bmat, dtype=np.float32)
    W_ro = np.asarray(W_ro, dtype=np.float32)

    jt = np.ascontiguousarray((HSCALE * DT * J).T).astype(np.float16)
    brow = (HSCALE * DT * Bmat[:, 0])[None, :].astype(np.float16)
    wrt = np.ascontiguousarray(W_ro[0].reshape(CB, 128).T).astype(np.float16)
    return [
        {
            "JT": jt,
            "brow": brow,
            "wrt": wrt,
            "velT": np.ascontiguousarray(
                vel[c * B_LOC : (c + 1) * B_LOC].T.reshape(1, -1)
            ).astype(np.float16),
        }
        for c in range(N_CORES)
    ]


def kernel(vel, J, Bmat, W_ro, _trace=False, **build_kw):
    from concourse.bass_utils import run_bass_kernel_spmd

    nc = _get_nc(**build_kw)
    in_maps = make_in_maps(vel, J, Bmat, W_ro)
    res = run_bass_kernel_spmd(nc, in_maps, list(range(N_CORES)), trace=_trace)
    # out[0, t*B_LOC + b] = readout(batch row b, step t)
    out = np.stack(
        [r["out"].reshape(T_FULL, B_LOC).T for r in res.results], axis=0
    ).reshape(BATCH, T_FULL)
    out = out[:, :, None].astype(np.float32)
    if _trace:
        kernel.last_results = res
    return out


kernel.last_results = None
